# revision 23
# baseline (speedup 1.0000x reference)
"""Trainium2 Bass kernel for the GRU autoencoder (v3).

Distribution (8 NeuronCores), as baseline:
  Encode : chain-parallel x batch-parallel. Core j handles GRU chain j//2
           (xf, xb, ef, eb) on batch half j%2 (BE=128 rows), uniform 100-step
           loop (x-chains padded with exact identity steps via BIG z-gate).
  Reshard: AllToAll (bf16) so core j decodes rows [16j:16j+16] u [128+16j:+16].
  Middle : per-shard MLP + decoder const precompute.
  Decode : 60 autoregressive steps on a 32-row shard.

v3 decode changes vs v2:
  * do->Wy matrix fusion: the y_p feedback path (do matmul -> y_sb -> ypT
    copy -> wy matmuls) is replaced by a host-composed Wfuse = do_W.T @ Wy.T
    applied directly to hm2, so the output head is off the critical path.
  * Step-0 gates (from x_last) fully precomputed on host (xg0).
  * const (cst) stays in SBUF as an id32-matmul stationary; no DRAM roundtrip.
  * omz = sigmoid(-g_z) via activation scale=-1; h' = omz*n + z*h with
    z*h and omz computed off-path on GpSimd while the n-path runs.
  * All decode shadow matmuls removed; PE gaps are filled with real work
    (next-step h-side gate pre-emit, output head).
"""

import sys

sys.path.insert(0, "/opt/trn_rl_repo")

import numpy as np

import concourse.bass as bass
import concourse.mybir as mybir
import concourse.tile as tile
from concourse import bacc
from concourse.masks import make_identity

dt = mybir.dt
AF = mybir.ActivationFunctionType
OP = mybir.AluOpType

B, TX, TY, NX, NY, H, HOR = 256, 50, 100, 64, 64, 512, 60
M1, M2 = 1024, 512
G = 3 * H
NCORE = 8
BE = 128   # encoder batch rows per core
BD = 32    # decoder batch rows per core
ET = 100   # uniform encoder step count
BIG = 30000.0

F32, BF16 = dt.float32, dt.bfloat16
BF = np.dtype(mybir.dt.np(BF16))


def build_nc(et=ET, hor=HOR):
    nc = bacc.Bacc("TRN2", target_bir_lowering=False, debug=False,
                   num_devices=NCORE)

    # ---- DRAM parameters (identical names on every core) -------------------
    d_xin = nc.dram_tensor("xin", [66, et * BE], BF16, kind="ExternalInput")
    d_wih = nc.dram_tensor("wih_aug", [66, G], BF16, kind="ExternalInput")
    d_whh = nc.dram_tensor("whh_t", [H, G], BF16, kind="ExternalInput")
    d_bhhn = nc.dram_tensor("bhhn_row", [1, H], BF16, kind="ExternalInput")

    d_dwhh = nc.dram_tensor("dwhh_blk", [H, G], BF16, kind="ExternalInput")
    d_wfz = nc.dram_tensor("wfz_blk", [M2, 1024], BF16, kind="ExternalInput")
    d_wfn = nc.dram_tensor("wfn_blk", [M2, 512], BF16, kind="ExternalInput")
    d_xg0 = nc.dram_tensor("xg0_blk", [BD, G], BF16, kind="ExternalInput")
    d_dbhhn = nc.dram_tensor("dbhhn_blk", [1, 512], BF16, kind="ExternalInput")
    d_dm1 = nc.dram_tensor("dm1_t", [H, M1], BF16, kind="ExternalInput")
    d_dm1b = nc.dram_tensor("dm1b_row", [1, M1], BF16, kind="ExternalInput")
    d_dm2 = nc.dram_tensor("dm2_t", [M1, M2], BF16, kind="ExternalInput")
    d_dm2b = nc.dram_tensor("dm2b_row", [1, M2], BF16, kind="ExternalInput")
    d_dow = nc.dram_tensor("dow_t", [M2, NY], BF16, kind="ExternalInput")
    d_dob = nc.dram_tensor("dob_row", [1, NY], BF16, kind="ExternalInput")

    d_em1x = nc.dram_tensor("em1x_t", [H, M1], BF16, kind="ExternalInput")
    d_em1y = nc.dram_tensor("em1y_t", [H, M1], BF16, kind="ExternalInput")
    d_em1b = nc.dram_tensor("em1b_row", [1, M1], BF16, kind="ExternalInput")
    d_em2 = nc.dram_tensor("em2_t", [M1, M2], BF16, kind="ExternalInput")
    d_em2b = nc.dram_tensor("em2b_row", [1, M2], BF16, kind="ExternalInput")
    d_eo = nc.dram_tensor("eo_t", [M2, H], BF16, kind="ExternalInput")
    d_eob = nc.dram_tensor("eob_row", [1, H], BF16, kind="ExternalInput")
    d_dcw = nc.dram_tensor("dcw_blk", [2 * H, G], BF16, kind="ExternalInput")
    d_dcb = nc.dram_tensor("dcb_blk", [1, G], BF16, kind="ExternalInput")

    d_out = nc.dram_tensor("out", [NY, hor * BD], F32, kind="ExternalOutput")

    cc_in = nc.dram_tensor("cc_in", [BE, H], BF16)
    cc_out = nc.dram_tensor("cc_out", [NCORE, 16, H], BF16)

    with tile.TileContext(nc) as tc:
        with tc.tile_pool(name="wts", bufs=1) as wp, \
             tc.tile_pool(name="xin", bufs=2) as xp, \
             tc.tile_pool(name="state", bufs=2) as st, \
             tc.tile_pool(name="chain", bufs=2) as ch, \
             tc.tile_pool(name="persist", bufs=1) as pe:

            # ---------- constants ----------
            idf = pe.tile([128, 128], F32, tag="idf")
            make_identity(nc, idf[:])
            idb = pe.tile([128, 128], BF16, tag="idb")
            nc.gpsimd.tensor_copy(idb[:], idf[:])
            ones_b = pe.tile([1, 128], BF16, tag="ones_b")
            nc.gpsimd.memset(ones_b[:], 1.0)

            def wload(dram_ap, rows, cols, tag):
                t = wp.tile([rows, cols], BF16, tag=tag)
                nc.sync.dma_start(t[:], dram_ap)
                return t

            # encoder weights first (needed immediately)
            wih = wload(d_wih[:], 66, G, "wih")
            whh = [wload(d_whh[128 * c:128 * (c + 1), :], 128, G, f"whh{c}")
                   for c in range(4)]
            bhhn = wload(d_bhhn[:], 1, H, "bhhn")

            # decoder/middle weights (DMA streams during encode)
            dwhh = [wload(d_dwhh[128 * c:128 * (c + 1), :], 128, G,
                          f"dwhh{c}") for c in range(4)]
            wfz = [wload(d_wfz[128 * c:128 * (c + 1), :], 128, 1024,
                         f"wfz{c}") for c in range(4)]
            wfn = [wload(d_wfn[128 * c:128 * (c + 1), :], 128, 512,
                         f"wfn{c}") for c in range(4)]
            xg0 = wload(d_xg0[:], BD, G, "xg0")
            dbhhn = wload(d_dbhhn[:], 1, 512, "dbhhn")
            dm1 = [wload(d_dm1[128 * c:128 * (c + 1), :], 128, M1,
                         f"dm1_{c}") for c in range(4)]
            dm1b = wload(d_dm1b[:], 1, M1, "dm1b")
            dm2 = [wload(d_dm2[128 * c:128 * (c + 1), :], 128, M2,
                         f"dm2_{c}") for c in range(8)]
            dm2b = wload(d_dm2b[:], 1, M2, "dm2b")
            dow = [wload(d_dow[128 * c:128 * (c + 1), :], 128, NY,
                         f"dow{c}") for c in range(4)]
            dob = wload(d_dob[:], 1, NY, "dob")
            em1x = [wload(d_em1x[128 * c:128 * (c + 1), :], 128, M1,
                          f"em1x{c}") for c in range(4)]
            em1y = [wload(d_em1y[128 * c:128 * (c + 1), :], 128, M1,
                          f"em1y{c}") for c in range(4)]
            em1b = wload(d_em1b[:], 1, M1, "em1b")
            em2 = [wload(d_em2[128 * c:128 * (c + 1), :], 128, M2,
                         f"em2_{c}") for c in range(8)]
            em2b = wload(d_em2b[:], 1, M2, "em2b")
            eo = [wload(d_eo[128 * c:128 * (c + 1), :], 128, H,
                        f"eo{c}") for c in range(4)]
            eob = wload(d_eob[:], 1, H, "eob")
            dcw = [wload(d_dcw[128 * c:128 * (c + 1), :], 128, G,
                         f"dcw{c}") for c in range(8)]
            dcb = wload(d_dcb[:], 1, G, "dcb")

            # ---------- encoder state ----------
            h_b = pe.tile([BE, H], BF16, tag="h0")
            nc.gpsimd.memset(h_b[:], 0.0)
            hT = pe.tile([128, H], BF16, tag="hT0")
            nc.gpsimd.memset(hT[:], 0.0)

            # ================= ENCODE =================
            with tc.tile_pool(name="pg3", bufs=2, space="PSUM") as pg3, \
                 tc.tile_pool(name="pg2", bufs=1, space="PSUM") as pg2, \
                 tc.tile_pool(name="pga", bufs=1, space="PSUM") as pga, \
                 tc.tile_pool(name="pgb", bufs=1, space="PSUM") as pgb, \
                 tc.tile_pool(name="ptr", bufs=2, space="PSUM") as ptr, \
                 tc.tile_pool(name="pwm", bufs=1, space="PSUM") as pwm:

                warm = pwm.tile([128, 512], F32, tag="warm")

                def shadow(tile_ap, n=1):
                    # PE filler pinned behind a chain op: a dummy matmul whose
                    # stationary operand is that op's output, so it runs right
                    # after it and holds the HAM clock gate at 8/8 through the
                    # elementwise window. No consumers.
                    for _ in range(n):
                        nc.tensor.matmul(warm[:], tile_ap, whh[0][:, 0:512],
                                         start=True, stop=True)

                for t in range(et):
                    if t % 4 == 0:
                        xb = xp.tile([66, 4 * BE], BF16, tag="xb")
                        nc.sync.dma_start(
                            xb[:], d_xin[:, t * BE:(t + 4) * BE])
                    xs = xb[:, (t % 4) * BE:(t % 4 + 1) * BE]

                    # PE order: g1a (r first), g3, g2 (n-path), g1b (z)
                    g1a = pga.tile([BE, 512], F32, tag="g1a")
                    nc.tensor.matmul(g1a[:], xs, wih[:, 0:512],
                                     start=True, stop=False)
                    for c in range(4):
                        nc.tensor.matmul(g1a[:], hT[:, 128 * c:128 * (c + 1)],
                                         whh[c][:, 0:512],
                                         start=False, stop=(c == 3))
                    # bias wave first: the bank opener with the weakest
                    # deps, so rhn never stalls on a late bias accumulate
                    g2 = pg2.tile([BE, 512], F32, tag="g2")
                    nc.tensor.matmul(g2[:], ones_b[0:1, 0:BE], bhhn[:],
                                     start=True, stop=False)
                    for c in range(4):
                        nc.tensor.matmul(g2[:], hT[:, 128 * c:128 * (c + 1)],
                                         whh[c][:, 1024:1536],
                                         start=False, stop=(c == 3))
                    g3 = pg3.tile([BE, 512], F32, tag="g3")
                    nc.tensor.matmul(g3[:], xs, wih[:, 1024:1536],
                                     start=True, stop=True)
                    g1b = pgb.tile([BE, 512], F32, tag="g1b")
                    nc.tensor.matmul(g1b[:], xs, wih[:, 512:1024],
                                     start=True, stop=False)
                    for c in range(4):
                        nc.tensor.matmul(g1b[:], hT[:, 128 * c:128 * (c + 1)],
                                         whh[c][:, 512:1024],
                                         start=False, stop=(c == 3))

                    # n-path half-split: half 1 trails half 0 by one stage
                    # across the Scalar/Vector engines
                    r_b = ch.tile([BE, 512], BF16, tag="r")
                    rhn = ch.tile([BE, 512], BF16, tag="rhn")
                    npre = ch.tile([BE, 512], BF16, tag="npre")
                    n_b = ch.tile([BE, 512], BF16, tag="n")
                    z_b = ch.tile([BE, 512], BF16, tag="z")
                    for half in range(2):
                        sl = slice(256 * half, 256 * (half + 1))
                        nc.scalar.activation(r_b[:, sl], g1a[:, sl],
                                             AF.Sigmoid)
                        nc.vector.tensor_mul(rhn[:, sl], r_b[:, sl],
                                             g2[:, sl])
                        shadow(r_b[:, 256 * half:256 * half + 128])
                    nc.scalar.activation(z_b[:], g1b[:], AF.Sigmoid)
                    for half in range(2):
                        sl = slice(256 * half, 256 * (half + 1))
                        nc.vector.tensor_add(npre[:, sl], rhn[:, sl],
                                             g3[:, sl])
                        nc.scalar.activation(n_b[:, sl], npre[:, sl], AF.Tanh)
                        if half == 0:
                            shadow(n_b[:, 0:128])
                    # h' = (1-z)*n + z*h; omz and zh are ready before tanh
                    # completes, so only mul+add remain on the critical path
                    omz = ch.tile([BE, 512], BF16, tag="omz")
                    nc.vector.tensor_scalar(omz[:], z_b[:],
                                            -1.0, 1.0, OP.mult, OP.add)
                    zh = ch.tile([BE, 512], BF16, tag="zh")
                    nc.vector.tensor_mul(zh[:], z_b[:], h_b[:])
                    t1 = ch.tile([BE, 512], BF16, tag="t1")
                    nc.vector.tensor_mul(t1[:], omz[:], n_b[:])
                    h_new = st.tile([BE, H], BF16, tag="h")
                    for half in range(2):
                        sl = slice(256 * half, 256 * (half + 1))
                        nc.vector.tensor_add(h_new[:, sl], t1[:, sl],
                                             zh[:, sl])
                    if t + 1 < et:
                        hT_new = st.tile([128, H], BF16, tag="hT")
                        for half in range(2):
                            p = ptr.tile([128, 256], BF16, tag="tr")
                            for c in range(2):
                                cc = 2 * half + c
                                nc.tensor.transpose(
                                    p[:, 128 * c:128 * (c + 1)],
                                    h_new[:, 128 * cc:128 * (cc + 1)], idb[:])
                            nc.vector.tensor_copy(
                                hT_new[:, 256 * half:256 * (half + 1)], p[:])
                        hT = hT_new
                    h_b = h_new

                nc.sync.dma_start(cc_in[:], h_b[:])

            # ================= RESHARD =================
            nc.gpsimd.collective_compute(
                "AllToAll", OP.bypass,
                replica_groups=[list(range(NCORE))],
                ins=[cc_in[:]], outs=[cc_out[:]])

            with tc.tile_pool(name="pgt", bufs=2, space="PSUM") as pgt, \
                 tc.tile_pool(name="pm1", bufs=1, space="PSUM") as pm1, \
                 tc.tile_pool(name="pm2", bufs=1, space="PSUM") as pm2, \
                 tc.tile_pool(name="ptr2", bufs=2, space="PSUM") as pt2, \
                 tc.tile_pool(name="pdo", bufs=1, space="PSUM") as pdo:

                # ---------- gather + h_x / h_y ----------
                pxa = ch.tile([BD, H], BF16, tag="pxa")
                pxb = ch.tile([BD, H], BF16, tag="pxb")
                pya = ch.tile([BD, H], BF16, tag="pya")
                pyb = ch.tile([BD, H], BF16, tag="pyb")
                nc.sync.dma_start(pxa[0:16, :], cc_out[0][:])
                nc.sync.dma_start(pxa[16:32, :], cc_out[1][:])
                nc.sync.dma_start(pxb[0:16, :], cc_out[2][:])
                nc.sync.dma_start(pxb[16:32, :], cc_out[3][:])
                nc.sync.dma_start(pya[0:16, :], cc_out[4][:])
                nc.sync.dma_start(pya[16:32, :], cc_out[5][:])
                nc.sync.dma_start(pyb[0:16, :], cc_out[6][:])
                nc.sync.dma_start(pyb[16:32, :], cc_out[7][:])
                hx = pe.tile([BD, H], BF16, tag="hx")
                hy = pe.tile([BD, H], BF16, tag="hy")
                nc.vector.tensor_add(hx[:], pxa[:], pxb[:])
                nc.vector.tensor_add(hy[:], pya[:], pyb[:])

                def trsp32(src_bf, tag):
                    """src [32, 512] (partition base 0) -> [128, 128] bf16,
                    h-dim chunk c -> cols 32c:32c+32."""
                    p = pt2.tile([128, 128], BF16, tag="trp")
                    for c in range(4):
                        nc.tensor.transpose(
                            p[:, 32 * c:32 * (c + 1)],
                            src_bf[0:32, 128 * c:128 * (c + 1)],
                            idb[0:32, 0:32])
                    o = pe.tile([128, 128], BF16, tag=tag)
                    nc.scalar.copy(o[:], p[:])
                    return o

                hxT = trsp32(hx, "hxT")
                hyT = trsp32(hy, "hyT")

                # ---------- m1 = relu(W1x hx + W1y hy + b1), blocked ------
                m1 = pm1.tile([128, 256], F32, tag="m1")
                for j in range(4):
                    for k in range(4):
                        nc.tensor.matmul(
                            m1[32 * j:32 * (j + 1), :], hxT[:, 32 * k:32 * (k + 1)],
                            em1x[k][:, 256 * j:256 * (j + 1)],
                            start=(k == 0), stop=False,
                            tile_position=(0, 32 * j))
                    for k in range(4):
                        nc.tensor.matmul(
                            m1[32 * j:32 * (j + 1), :], hyT[:, 32 * k:32 * (k + 1)],
                            em1y[k][:, 256 * j:256 * (j + 1)],
                            start=False, stop=False,
                            tile_position=(0, 32 * j))
                    nc.tensor.matmul(
                        m1[32 * j:32 * (j + 1), :], ones_b[0:1, 0:BD],
                        em1b[:, 256 * j:256 * (j + 1)], start=False, stop=True,
                        tile_position=(0, 32 * j))
                hm1 = ch.tile([128, 256], BF16, tag="hm1")
                nc.scalar.activation(hm1[:], m1[:], AF.Relu)

                def tr_m1(src_bf, tag):
                    """blocked [ (j,b32), 256 ] -> [128, 32*8] chunks of the
                    1024-dim; chunk k at dst col 32*perm[k]. Two full-tile
                    half transposes: left half -> even chunks, right -> odd."""
                    p = pt2.tile([128, 256], BF16, tag="trp")
                    nc.tensor.transpose(p[:, 0:128], src_bf[:, 0:128], idb[:])
                    nc.tensor.transpose(p[:, 128:256], src_bf[:, 128:256],
                                        idb[:])
                    perm = [0, 4, 1, 5, 2, 6, 3, 7]
                    o = pe.tile([128, 256], BF16, tag=tag)
                    nc.scalar.copy(o[:], p[:])
                    return o, perm

                hm1T, p1 = tr_m1(hm1, "hm1T")

                # ---------- m2 = relu(W2 m1 + b2), blocked ----------------
                m2 = pm2.tile([128, 128], F32, tag="m2")
                for j in range(4):
                    for k in range(8):
                        nc.tensor.matmul(
                            m2[32 * j:32 * (j + 1), :],
                            hm1T[:, 32 * p1[k]:32 * (p1[k] + 1)],
                            em2[k][:, 128 * j:128 * (j + 1)],
                            start=(k == 0), stop=False,
                            tile_position=(0, 32 * j))
                    nc.tensor.matmul(
                        m2[32 * j:32 * (j + 1), :], ones_b[0:1, 0:BD],
                        em2b[:, 128 * j:128 * (j + 1)], start=False, stop=True,
                        tile_position=(0, 32 * j))
                hm2 = ch.tile([128, 128], BF16, tag="hm2")
                nc.scalar.activation(hm2[:], m2[:], AF.Relu)

                def tr_sq(src_bf, tag):
                    """blocked [(j,b32), 128] -> [128, 128]; chunk j -> col 32j.
                    One full-tile transpose: out[:, 32j+b] = src[32j+b, :]."""
                    p = pt2.tile([128, 128], BF16, tag="trp")
                    nc.tensor.transpose(p[:], src_bf[:], idb[:])
                    o = pe.tile([128, 128], BF16, tag=tag)
                    nc.scalar.copy(o[:], p[:])
                    return o

                hm2T = tr_sq(hm2, "hm2T")

                # ---------- z = eo m2 + b, blocked ------------------------
                zp = pm2.tile([128, 128], F32, tag="m2")
                for j in range(4):
                    for k in range(4):
                        nc.tensor.matmul(
                            zp[32 * j:32 * (j + 1), :],
                            hm2T[:, 32 * k:32 * (k + 1)],
                            eo[k][:, 128 * j:128 * (j + 1)],
                            start=(k == 0), stop=False,
                            tile_position=(0, 32 * j))
                    nc.tensor.matmul(
                        zp[32 * j:32 * (j + 1), :], ones_b[0:1, 0:BD],
                        eob[:, 128 * j:128 * (j + 1)], start=False, stop=True,
                        tile_position=(0, 32 * j))
                z_b = ch.tile([128, 128], BF16, tag="z_b")
                nc.scalar.copy(z_b[:], zp[:])
                zT = tr_sq(z_b, "zT")

                # ---------- cst = dcw [hx; z] + dcb, blocked --------------
                # cst stays in SBUF: per-row constant for every decode step's
                # gates, applied via id32-stationary matmuls.
                cstp = pgt.tile([128, 512], F32, tag="g")
                for j in range(4):
                    for k in range(4):
                        nc.tensor.matmul(
                            cstp[32 * j:32 * (j + 1), 0:384],
                            hxT[:, 32 * k:32 * (k + 1)],
                            dcw[k][:, 384 * j:384 * (j + 1)],
                            start=(k == 0), stop=False,
                            tile_position=(0, 32 * j))
                    for k in range(4):
                        nc.tensor.matmul(
                            cstp[32 * j:32 * (j + 1), 0:384],
                            zT[:, 32 * k:32 * (k + 1)],
                            dcw[4 + k][:, 384 * j:384 * (j + 1)],
                            start=False, stop=False,
                            tile_position=(0, 32 * j))
                    nc.tensor.matmul(
                        cstp[32 * j:32 * (j + 1), 0:384], ones_b[0:1, 0:BD],
                        dcb[:, 384 * j:384 * (j + 1)], start=False, stop=True,
                        tile_position=(0, 32 * j))
                # cst stored [32, 4*384]: block j at cols 384j (partition base
                # 0, so it can be a matmul stationary alongside id32).
                cst_b = pe.tile([BD, G], BF16, tag="cst")
                for j in range(4):
                    nc.scalar.copy(cst_b[0:32, 384 * j:384 * (j + 1)],
                                   cstp[32 * j:32 * (j + 1), 0:384])

                # ---------- decoder init ----------
                hd_b = st.tile([128, 128], BF16, tag="hd")
                nc.gpsimd.memset(hd_b[:], 0.0)

                id32 = idb[0:32, 0:32]

                def emit_const(g):
                    """cst contribution: rz part -> cols 0:256 (accumulate),
                    bih_n part -> cols 384:512 (starts that region)."""
                    for j in range(4):
                        nc.tensor.matmul(
                            g[32 * j:32 * (j + 1), 0:256], id32,
                            cst_b[0:32, 384 * j:384 * j + 256],
                            start=False, stop=False,
                            tile_position=(0, 32 * j))
                    for j in range(4):
                        nc.tensor.matmul(
                            g[32 * j:32 * (j + 1), 384:512], id32,
                            cst_b[0:32, 384 * j + 256:384 * (j + 1)],
                            start=True, stop=False,
                            tile_position=(0, 32 * j))

                def emit_hside(g, hdT_src):
                    """Whh (cols 0:384, starts 0:384) + bhh_n bias (stops
                    256:384)."""
                    for k in range(4):
                        for j in range(4):
                            nc.tensor.matmul(
                                g[32 * j:32 * (j + 1), 0:384],
                                hdT_src[:, 32 * k:32 * (k + 1)],
                                dwhh[k][:, 384 * j:384 * (j + 1)],
                                start=(k == 0), stop=False,
                                tile_position=(0, 32 * j))
                    for j in range(4):
                        nc.tensor.matmul(
                            g[32 * j:32 * (j + 1), 256:384], ones_b[0:1, 0:BD],
                            dbhhn[:, 128 * j:128 * (j + 1)],
                            start=False, stop=True,
                            tile_position=(0, 32 * j))

                def emit_fused(g, hm2T_src):
                    """hm2-side (fused do->Wy): rz waves first (close 0:256),
                    then xn waves (close 384:512)."""
                    for k in range(4):
                        for j in range(4):
                            nc.tensor.matmul(
                                g[32 * j:32 * (j + 1), 0:256],
                                hm2T_src[:, 32 * k:32 * (k + 1)],
                                wfz[k][:, 256 * j:256 * (j + 1)],
                                start=False, stop=(k == 3),
                                tile_position=(0, 32 * j))
                    for k in range(4):
                        for j in range(4):
                            nc.tensor.matmul(
                                g[32 * j:32 * (j + 1), 384:512],
                                hm2T_src[:, 32 * k:32 * (k + 1)],
                                wfn[k][:, 128 * j:128 * (j + 1)],
                                start=False, stop=(k == 3),
                                tile_position=(0, 32 * j))

                P1 = [0, 4, 1, 5, 2, 6, 3, 7]

                # ---- step-0 gates: cst + x_last-side (host-precomputed) ----
                g_cur = pgt.tile([128, 512], F32, tag="g")
                for j in range(4):
                    nc.tensor.matmul(
                        g_cur[32 * j:32 * (j + 1), 0:256], id32,
                        cst_b[0:32, 384 * j:384 * j + 256],
                        start=True, stop=False, tile_position=(0, 32 * j))
                for j in range(4):
                    nc.tensor.matmul(
                        g_cur[32 * j:32 * (j + 1), 0:256], id32,
                        xg0[0:32, 384 * j:384 * j + 256],
                        start=False, stop=True, tile_position=(0, 32 * j))
                for j in range(4):
                    nc.tensor.matmul(
                        g_cur[32 * j:32 * (j + 1), 384:512], id32,
                        cst_b[0:32, 384 * j + 256:384 * (j + 1)],
                        start=True, stop=False, tile_position=(0, 32 * j))
                for j in range(4):
                    nc.tensor.matmul(
                        g_cur[32 * j:32 * (j + 1), 384:512], id32,
                        xg0[0:32, 384 * j + 256:384 * (j + 1)],
                        start=False, stop=True, tile_position=(0, 32 * j))
                for j in range(4):
                    nc.tensor.matmul(
                        g_cur[32 * j:32 * (j + 1), 256:384], ones_b[0:1, 0:BD],
                        dbhhn[:, 128 * j:128 * (j + 1)],
                        start=True, stop=True, tile_position=(0, 32 * j))

                # ================= DECODE =================
                for t in range(hor):
                    g = g_cur
                    # ---- GRU cell elementwise ----
                    rz = ch.tile([128, 256], BF16, tag="rz")
                    nc.scalar.activation(rz[:], g[:, 0:256], AF.Sigmoid)
                    # off-path (GpSimd): omz = 1-z = sigmoid(-g_z); zh = z*h
                    omz = ch.tile([128, 128], BF16, tag="omz")
                    nc.vector.tensor_scalar(omz[:], rz[:, 128:256],
                                            -1.0, 1.0, OP.mult, OP.add)
                    zh = ch.tile([128, 128], BF16, tag="zh")
                    nc.vector.tensor_mul(zh[:], rz[:, 128:256], hd_b[:])
                    # n-path
                    rhn = ch.tile([128, 128], BF16, tag="drhn")
                    nc.vector.tensor_mul(rhn[:], rz[:, 0:128], g[:, 256:384])
                    npre = ch.tile([128, 128], BF16, tag="dnpre")
                    nc.vector.tensor_add(npre[:], rhn[:], g[:, 384:512])
                    n_b = ch.tile([128, 128], BF16, tag="dn")
                    nc.scalar.activation(n_b[:], npre[:], AF.Tanh)
                    # h' = omz*n + zh
                    t1 = ch.tile([128, 128], BF16, tag="dt1")
                    nc.vector.tensor_mul(t1[:], omz[:], n_b[:])
                    hd_new = st.tile([128, 128], BF16, tag="hd")
                    nc.vector.tensor_add(hd_new[:], t1[:], zh[:])

                    ptd = pt2.tile([128, 128], BF16, tag="trp")
                    nc.tensor.transpose(ptd[:], hd_new[:], idb[:])
                    hdT = st.tile([128, 128], BF16, tag="hdT")
                    nc.scalar.copy(hdT[:], ptd[:])
                    hd_b = hd_new

                    # ---- m1 ----
                    m1 = pm1.tile([128, 256], F32, tag="m1")
                    for k in range(4):
                        for j in range(4):
                            nc.tensor.matmul(
                                m1[32 * j:32 * (j + 1), :],
                                hdT[:, 32 * k:32 * (k + 1)],
                                dm1[k][:, 256 * j:256 * (j + 1)],
                                start=(k == 0), stop=False,
                                tile_position=(0, 32 * j))
                    for j in range(4):
                        nc.tensor.matmul(
                            m1[32 * j:32 * (j + 1), :], ones_b[0:1, 0:BD],
                            dm1b[:, 256 * j:256 * (j + 1)],
                            start=False, stop=True,
                            tile_position=(0, 32 * j))
                    # pre-emit next step's h-side gates: they stream on PE
                    # while this step's m1 relu/transpose waits on Scalar.
                    if t + 1 < hor:
                        g_cur = pgt.tile([128, 512], F32, tag="g")
                        emit_hside(g_cur, hdT)
                        emit_const(g_cur)

                    hm1 = ch.tile([128, 256], BF16, tag="hm1")
                    p1t = pt2.tile([128, 256], BF16, tag="trp")
                    hm1T = ch.tile([128, 256], BF16, tag="hm1T")
                    for hf in range(2):
                        sl = slice(128 * hf, 128 * (hf + 1))
                        nc.scalar.activation(hm1[:, sl], m1[:, sl], AF.Relu)
                        nc.tensor.transpose(p1t[:, sl], hm1[:, sl], idb[:])
                        nc.vector.tensor_copy(hm1T[:, sl], p1t[:, sl])

                    # ---- m2 (even chunks first: left half of hm1T) ----
                    m2 = pm2.tile([128, 128], F32, tag="m2")
                    for ki, k in enumerate([0, 2, 4, 6, 1, 3, 5, 7]):
                        for j in range(4):
                            nc.tensor.matmul(
                                m2[32 * j:32 * (j + 1), :],
                                hm1T[:, 32 * P1[k]:32 * (P1[k] + 1)],
                                dm2[k][:, 128 * j:128 * (j + 1)],
                                start=(ki == 0), stop=False,
                                tile_position=(0, 32 * j))
                    for j in range(4):
                        nc.tensor.matmul(
                            m2[32 * j:32 * (j + 1), :], ones_b[0:1, 0:BD],
                            dm2b[:, 128 * j:128 * (j + 1)],
                            start=False, stop=True,
                            tile_position=(0, 32 * j))
                    hm2 = ch.tile([128, 128], BF16, tag="hm2")
                    nc.scalar.activation(hm2[:], m2[:], AF.Relu)
                    p2t = pt2.tile([128, 128], BF16, tag="trp")
                    nc.tensor.transpose(p2t[:], hm2[:], idb[:])
                    hm2T = ch.tile([128, 128], BF16, tag="hm2T")
                    nc.vector.tensor_copy(hm2T[:], p2t[:])

                    # ---- fused y-side of next step's gates (critical path) --
                    if t + 1 < hor:
                        emit_fused(g_cur, hm2T)

                    # ---- output head (off critical path) ----
                    ytp = pdo.tile([NY, BD], F32, tag="do")
                    for k in range(4):
                        nc.tensor.matmul(ytp[:], dow[k][:, 0:NY],
                                         hm2T[:, 32 * k:32 * (k + 1)],
                                         start=(k == 0), stop=False)
                    nc.tensor.matmul(ytp[:], dob[:], ones_b[0:1, 0:BD],
                                     start=False, stop=True)
                    y_sb = ch.tile([NY, BD], F32, tag="y_sb")
                    nc.vector.tensor_copy(y_sb[:], ytp[:])
                    nc.sync.dma_start(d_out[:, BD * t:BD * (t + 1)], y_sb[:])

    nc.compile()
    return nc


# ---------------------------------------------------------------------------
# Host-side sharding
# ---------------------------------------------------------------------------

# gate-block permutation: blocked col j*384 + s*128 + c  <- gate row 512s+128j+c
_IDXG = np.array([512 * s + 128 * j + c
                  for j in range(4) for s in range(3) for c in range(128)])
_IDXRZ = np.array([512 * s + 128 * j + c
                   for j in range(4) for s in range(2) for c in range(128)])
_IDXN = np.array([1024 + 128 * j + c for j in range(4) for c in range(128)])
# step-0 gate blocked layout: per block j, [rz(256) | xn(128)]
_IDXG0 = np.array([(512 * (u // 128) + 128 * j + (u % 128)) if u < 256
                   else (1024 + 128 * j + (u - 256))
                   for j in range(4) for u in range(384)])


def shard_inputs(inp, et=ET, hor=HOR):
    f32 = np.float32
    x, y = np.asarray(inp["x"], f32), np.asarray(inp["y"], f32)
    chains = [("xf", False, x), ("xb", True, x),
              ("ef", False, y), ("eb", True, y)]
    in_maps = []
    shared = {}

    def bf(a):
        return np.ascontiguousarray(np.asarray(a, f32).astype(BF))

    def wih_aug(pre):
        wih = np.asarray(inp[pre + "_Wih"], f32)
        bih = np.asarray(inp[pre + "_bih"], f32)
        bhh = np.asarray(inp[pre + "_bhh"], f32)
        aug = np.zeros((66, G), f32)
        aug[0:64, :] = wih.T
        bias = bih.copy()
        bias[0:2 * H] += bhh[0:2 * H]
        aug[64, :] = bias
        aug[65, H:2 * H] = BIG
        return bf(aug)

    d_Wih = np.asarray(inp["d_Wih"], f32)
    d_bih = np.asarray(inp["d_bih"], f32)
    d_bhh = np.asarray(inp["d_bhh"], f32)
    do_W = np.asarray(inp["do_W"], f32)
    do_b = np.asarray(inp["do_b"], f32)

    # decoder GRU weights, gate-blocked
    shared["dwhh_blk"] = bf(np.asarray(inp["d_Whh"], f32).T[:, _IDXG])
    WyT = d_Wih[:, 2 * H:].T                      # [64, 1536]
    # fused do->Wy: y_p-side of gates = hm2 @ Wfuse + bfuse
    Wfuse = do_W.T @ WyT                           # [512, 1536]
    bfuse = do_b @ WyT                             # [1536]
    shared["wfz_blk"] = bf(Wfuse[:, _IDXRZ])
    shared["wfn_blk"] = bf(Wfuse[:, _IDXN])
    shared["dbhhn_blk"] = bf(d_bhh[None, 2 * H:])
    dcb = d_bih + bfuse
    dcb[0:2 * H] += d_bhh[0:2 * H]
    shared["dcw_blk"] = bf(d_Wih[:, 0:2 * H].T[:, _IDXG])
    shared["dcb_blk"] = bf(dcb[None, _IDXG])

    shared["dm1_t"] = bf(np.asarray(inp["dm_W1"], f32).T)
    shared["dm1b_row"] = bf(np.asarray(inp["dm_b1"], f32)[None, :])
    shared["dm2_t"] = bf(np.asarray(inp["dm_W2"], f32).T)
    shared["dm2b_row"] = bf(np.asarray(inp["dm_b2"], f32)[None, :])
    shared["dow_t"] = bf(do_W.T)
    shared["dob_row"] = bf(do_b[None, :])

    em_W1 = np.asarray(inp["em_W1"], f32)
    shared["em1x_t"] = bf(em_W1[:, 0:H].T)
    shared["em1y_t"] = bf(em_W1[:, H:].T)
    shared["em1b_row"] = bf(np.asarray(inp["em_b1"], f32)[None, :])
    shared["em2_t"] = bf(np.asarray(inp["em_W2"], f32).T)
    shared["em2b_row"] = bf(np.asarray(inp["em_b2"], f32)[None, :])
    shared["eo_t"] = bf(np.asarray(inp["eo_W"], f32).T)
    shared["eob_row"] = bf(np.asarray(inp["eo_b"], f32)[None, :])

    for j in range(NCORE):
        chain, half = j // 2, j % 2
        pre, rev, seq = chains[chain]
        T = seq.shape[1]
        s = seq[128 * half:128 * (half + 1)]          # [128, T, 64]
        xin = np.zeros((66, et, BE), f32)
        xin[64, :, :] = 1.0
        pad = et - T
        if pad:
            xin[65, 0:pad, :] = 1.0
        order = np.arange(T)[::-1] if rev else np.arange(T)
        xin[0:64, pad:, :] = s[:, order, :].transpose(2, 1, 0)
        m = dict(shared)
        m["xin"] = bf(xin.reshape(66, et * BE))
        m["wih_aug"] = wih_aug(pre)
        m["whh_t"] = bf(np.asarray(inp[pre + "_Whh"], f32).T)
        m["bhhn_row"] = bf(np.asarray(inp[pre + "_bhh"], f32)[None, 2 * H:])
        xl = np.concatenate([x[16 * j:16 * j + 16, -1, :],
                             x[128 + 16 * j:128 + 16 * j + 16, -1, :]])
        # bfuse is folded into cst (applied every step), but step 0's y-side
        # is x_last directly (no do_b) -> pre-subtract it here.
        xg0 = xl @ WyT - bfuse                        # [32, 1536]
        m["xg0_blk"] = bf(xg0[:, _IDXG0])
        in_maps.append(m)
    return in_maps


def unshard(results, hor=HOR):
    out = np.zeros((B, hor, NY), np.float32)
    for j in range(NCORE):
        o = results[j]["out"].reshape(NY, hor, BD).transpose(2, 1, 0)
        out[16 * j:16 * j + 16] = o[0:16]
        out[128 + 16 * j:128 + 16 * j + 16] = o[16:32]
    return out


_NC = None


def kernel(**inputs):
    global _NC
    from concourse.bass_utils import run_bass_kernel_spmd
    if _NC is None:
        _NC = build_nc()
    in_maps = shard_inputs(inputs)
    res = run_bass_kernel_spmd(_NC, in_maps, core_ids=list(range(NCORE)))
    return unshard(res.results)


# revision 24
# speedup vs baseline: 1.2370x; 1.2370x over previous
"""Trainium2 Bass kernel for the GRU autoencoder (v3).

Distribution (8 NeuronCores), as baseline:
  Encode : chain-parallel x batch-parallel. Core j handles GRU chain j//2
           (xf, xb, ef, eb) on batch half j%2 (BE=128 rows), uniform 100-step
           loop (x-chains padded with exact identity steps via BIG z-gate).
  Reshard: AllToAll (bf16) so core j decodes rows [16j:16j+16] u [128+16j:+16].
  Middle : per-shard MLP + decoder const precompute.
  Decode : 60 autoregressive steps on a 32-row shard.

v3 decode changes vs v2:
  * do->Wy matrix fusion: the y_p feedback path (do matmul -> y_sb -> ypT
    copy -> wy matmuls) is replaced by a host-composed Wfuse = do_W.T @ Wy.T
    applied directly to hm2, so the output head is off the critical path.
  * Step-0 gates (from x_last) fully precomputed on host (xg0).
  * const (cst) stays in SBUF as an id32-matmul stationary; no DRAM roundtrip.
  * omz = sigmoid(-g_z) via activation scale=-1; h' = omz*n + z*h with
    z*h and omz computed off-path on GpSimd while the n-path runs.
  * All decode shadow matmuls removed; PE gaps are filled with real work
    (next-step h-side gate pre-emit, output head).
"""

import sys

sys.path.insert(0, "/opt/trn_rl_repo")

import numpy as np

import concourse.bass as bass
import concourse.mybir as mybir
import concourse.tile as tile
from concourse import bacc
from concourse.masks import make_identity

dt = mybir.dt
AF = mybir.ActivationFunctionType
OP = mybir.AluOpType

B, TX, TY, NX, NY, H, HOR = 256, 50, 100, 64, 64, 512, 60
M1, M2 = 1024, 512
G = 3 * H
NCORE = 8
BE = 128   # encoder batch rows per core
BD = 32    # decoder batch rows per core
ET = 100   # uniform encoder step count
BIG = 30000.0

F32, BF16 = dt.float32, dt.bfloat16
BF = np.dtype(mybir.dt.np(BF16))


def build_nc(et=ET, hor=HOR):
    nc = bacc.Bacc("TRN2", target_bir_lowering=False, debug=False,
                   num_devices=NCORE)

    # ---- DRAM parameters (identical names on every core) -------------------
    d_xin = nc.dram_tensor("xin", [66, et * BE], BF16, kind="ExternalInput")
    d_wih = nc.dram_tensor("wih_aug", [66, G], BF16, kind="ExternalInput")
    d_whh = nc.dram_tensor("whh_t", [H, G], BF16, kind="ExternalInput")
    d_bhhn = nc.dram_tensor("bhhn_row", [1, H], BF16, kind="ExternalInput")

    d_dwhh = nc.dram_tensor("dwhh_blk", [H, G], BF16, kind="ExternalInput")
    d_wfz = nc.dram_tensor("wfz_blk", [M2, 1024], BF16, kind="ExternalInput")
    d_wfn = nc.dram_tensor("wfn_blk", [M2, 512], BF16, kind="ExternalInput")
    d_xg0 = nc.dram_tensor("xg0_blk", [BD, G], BF16, kind="ExternalInput")
    d_dbhhn = nc.dram_tensor("dbhhn_blk", [1, 512], BF16, kind="ExternalInput")
    d_dm1 = nc.dram_tensor("dm1_t", [H, M1], BF16, kind="ExternalInput")
    d_dm1b = nc.dram_tensor("dm1b_row", [1, M1], BF16, kind="ExternalInput")
    d_dm2 = nc.dram_tensor("dm2_t", [M1, M2], BF16, kind="ExternalInput")
    d_dm2b = nc.dram_tensor("dm2b_row", [1, M2], BF16, kind="ExternalInput")
    d_dow = nc.dram_tensor("dow_t", [M2, NY], BF16, kind="ExternalInput")
    d_dob = nc.dram_tensor("dob_row", [1, NY], BF16, kind="ExternalInput")

    d_em1x = nc.dram_tensor("em1x_t", [H, M1], BF16, kind="ExternalInput")
    d_em1y = nc.dram_tensor("em1y_t", [H, M1], BF16, kind="ExternalInput")
    d_em1b = nc.dram_tensor("em1b_row", [1, M1], BF16, kind="ExternalInput")
    d_em2 = nc.dram_tensor("em2_t", [M1, M2], BF16, kind="ExternalInput")
    d_em2b = nc.dram_tensor("em2b_row", [1, M2], BF16, kind="ExternalInput")
    d_eo = nc.dram_tensor("eo_t", [M2, H], BF16, kind="ExternalInput")
    d_eob = nc.dram_tensor("eob_row", [1, H], BF16, kind="ExternalInput")
    d_dcw = nc.dram_tensor("dcw_blk", [2 * H, G], BF16, kind="ExternalInput")
    d_dcb = nc.dram_tensor("dcb_blk", [1, G], BF16, kind="ExternalInput")

    d_out = nc.dram_tensor("out", [NY, hor * BD], F32, kind="ExternalOutput")

    cc_in = nc.dram_tensor("cc_in", [BE, H], BF16)
    cc_out = nc.dram_tensor("cc_out", [NCORE, 16, H], BF16)

    with tile.TileContext(nc) as tc:
        with tc.tile_pool(name="wts", bufs=1) as wp, \
             tc.tile_pool(name="xin", bufs=2) as xp, \
             tc.tile_pool(name="state", bufs=2) as st, \
             tc.tile_pool(name="chain", bufs=2) as ch, \
             tc.tile_pool(name="persist", bufs=1) as pe:

            # ---------- constants ----------
            idf = pe.tile([128, 128], F32, tag="idf")
            make_identity(nc, idf[:])
            idb = pe.tile([128, 128], BF16, tag="idb")
            nc.gpsimd.tensor_copy(idb[:], idf[:])
            ones_b = pe.tile([1, 128], BF16, tag="ones_b")
            nc.gpsimd.memset(ones_b[:], 1.0)

            def wload(dram_ap, rows, cols, tag):
                t = wp.tile([rows, cols], BF16, tag=tag)
                nc.sync.dma_start(t[:], dram_ap)
                return t

            # encoder weights first (needed immediately)
            wih = wload(d_wih[:], 66, G, "wih")
            whh = [wload(d_whh[128 * c:128 * (c + 1), :], 128, G, f"whh{c}")
                   for c in range(4)]
            bhhn = wload(d_bhhn[:], 1, H, "bhhn")

            # decoder/middle weights (DMA streams during encode)
            dwhh = [wload(d_dwhh[128 * c:128 * (c + 1), :], 128, G,
                          f"dwhh{c}") for c in range(4)]
            wfz = [wload(d_wfz[128 * c:128 * (c + 1), :], 128, 1024,
                         f"wfz{c}") for c in range(4)]
            wfn = [wload(d_wfn[128 * c:128 * (c + 1), :], 128, 512,
                         f"wfn{c}") for c in range(4)]
            xg0 = wload(d_xg0[:], BD, G, "xg0")
            dbhhn = wload(d_dbhhn[:], 1, 512, "dbhhn")
            dm1 = [wload(d_dm1[128 * c:128 * (c + 1), :], 128, M1,
                         f"dm1_{c}") for c in range(4)]
            dm1b = wload(d_dm1b[:], 1, M1, "dm1b")
            dm2 = [wload(d_dm2[128 * c:128 * (c + 1), :], 128, M2,
                         f"dm2_{c}") for c in range(8)]
            dm2b = wload(d_dm2b[:], 1, M2, "dm2b")
            dow = [wload(d_dow[128 * c:128 * (c + 1), :], 128, NY,
                         f"dow{c}") for c in range(4)]
            dob = wload(d_dob[:], 1, NY, "dob")
            em1x = [wload(d_em1x[128 * c:128 * (c + 1), :], 128, M1,
                          f"em1x{c}") for c in range(4)]
            em1y = [wload(d_em1y[128 * c:128 * (c + 1), :], 128, M1,
                          f"em1y{c}") for c in range(4)]
            em1b = wload(d_em1b[:], 1, M1, "em1b")
            em2 = [wload(d_em2[128 * c:128 * (c + 1), :], 128, M2,
                         f"em2_{c}") for c in range(8)]
            em2b = wload(d_em2b[:], 1, M2, "em2b")
            eo = [wload(d_eo[128 * c:128 * (c + 1), :], 128, H,
                        f"eo{c}") for c in range(4)]
            eob = wload(d_eob[:], 1, H, "eob")
            dcw = [wload(d_dcw[128 * c:128 * (c + 1), :], 128, G,
                         f"dcw{c}") for c in range(8)]
            dcb = wload(d_dcb[:], 1, G, "dcb")

            # ---------- encoder state ----------
            h_b = pe.tile([BE, H], BF16, tag="h0")
            nc.gpsimd.memset(h_b[:], 0.0)
            hT = pe.tile([128, H], BF16, tag="hT0")
            nc.gpsimd.memset(hT[:], 0.0)

            # ================= ENCODE =================
            with tc.tile_pool(name="pg3", bufs=2, space="PSUM") as pg3, \
                 tc.tile_pool(name="pg2", bufs=1, space="PSUM") as pg2, \
                 tc.tile_pool(name="pga", bufs=1, space="PSUM") as pga, \
                 tc.tile_pool(name="pgb", bufs=1, space="PSUM") as pgb, \
                 tc.tile_pool(name="ptr", bufs=2, space="PSUM") as ptr, \
                 tc.tile_pool(name="pwm", bufs=1, space="PSUM") as pwm:

                warm = pwm.tile([128, 512], F32, tag="warm")

                def shadow(tile_ap, n=1):
                    # PE filler pinned behind a chain op: a dummy matmul whose
                    # stationary operand is that op's output, so it runs right
                    # after it and holds the HAM clock gate at 8/8 through the
                    # elementwise window. No consumers.
                    for _ in range(n):
                        nc.tensor.matmul(warm[:], tile_ap, whh[0][:, 0:512],
                                         start=True, stop=True)

                for t in range(et):
                    if t % 4 == 0:
                        xb = xp.tile([66, 4 * BE], BF16, tag="xb")
                        nc.sync.dma_start(
                            xb[:], d_xin[:, t * BE:(t + 4) * BE])
                    xs = xb[:, (t % 4) * BE:(t % 4 + 1) * BE]

                    # PE order: g1a (r first), g3, g2 (n-path), g1b (z)
                    g1a = pga.tile([BE, 512], F32, tag="g1a")
                    nc.tensor.matmul(g1a[:], xs, wih[:, 0:512],
                                     start=True, stop=False)
                    for c in range(4):
                        nc.tensor.matmul(g1a[:], hT[:, 128 * c:128 * (c + 1)],
                                         whh[c][:, 0:512],
                                         start=False, stop=(c == 3))
                    # bias wave first: the bank opener with the weakest
                    # deps, so rhn never stalls on a late bias accumulate
                    g2 = pg2.tile([BE, 512], F32, tag="g2")
                    nc.tensor.matmul(g2[:], ones_b[0:1, 0:BE], bhhn[:],
                                     start=True, stop=False)
                    for c in range(4):
                        nc.tensor.matmul(g2[:], hT[:, 128 * c:128 * (c + 1)],
                                         whh[c][:, 1024:1536],
                                         start=False, stop=(c == 3))
                    g3 = pg3.tile([BE, 512], F32, tag="g3")
                    nc.tensor.matmul(g3[:], xs, wih[:, 1024:1536],
                                     start=True, stop=True)
                    g1b = pgb.tile([BE, 512], F32, tag="g1b")
                    nc.tensor.matmul(g1b[:], xs, wih[:, 512:1024],
                                     start=True, stop=False)
                    for c in range(4):
                        nc.tensor.matmul(g1b[:], hT[:, 128 * c:128 * (c + 1)],
                                         whh[c][:, 512:1024],
                                         start=False, stop=(c == 3))

                    # n-path half-split: half 1 trails half 0 by one stage
                    # across the Scalar/Vector engines
                    r_b = ch.tile([BE, 512], BF16, tag="r")
                    rhn = ch.tile([BE, 512], BF16, tag="rhn")
                    npre = ch.tile([BE, 512], BF16, tag="npre")
                    n_b = ch.tile([BE, 512], BF16, tag="n")
                    z_b = ch.tile([BE, 512], BF16, tag="z")
                    for half in range(2):
                        sl = slice(256 * half, 256 * (half + 1))
                        nc.scalar.activation(r_b[:, sl], g1a[:, sl],
                                             AF.Sigmoid)
                        nc.vector.tensor_mul(rhn[:, sl], r_b[:, sl],
                                             g2[:, sl])
                        shadow(r_b[:, 256 * half:256 * half + 128])
                    nc.scalar.activation(z_b[:], g1b[:], AF.Sigmoid)
                    for half in range(2):
                        sl = slice(256 * half, 256 * (half + 1))
                        nc.vector.tensor_add(npre[:, sl], rhn[:, sl],
                                             g3[:, sl])
                        nc.scalar.activation(n_b[:, sl], npre[:, sl], AF.Tanh)
                        if half == 0:
                            shadow(n_b[:, 0:128])
                    # h' = (1-z)*n + z*h; omz and zh are ready before tanh
                    # completes, so only mul+add remain on the critical path
                    omz = ch.tile([BE, 512], BF16, tag="omz")
                    nc.vector.tensor_scalar(omz[:], z_b[:],
                                            -1.0, 1.0, OP.mult, OP.add)
                    zh = ch.tile([BE, 512], BF16, tag="zh")
                    nc.vector.tensor_mul(zh[:], z_b[:], h_b[:])
                    t1 = ch.tile([BE, 512], BF16, tag="t1")
                    nc.vector.tensor_mul(t1[:], omz[:], n_b[:])
                    h_new = st.tile([BE, H], BF16, tag="h")
                    for half in range(2):
                        sl = slice(256 * half, 256 * (half + 1))
                        nc.vector.tensor_add(h_new[:, sl], t1[:, sl],
                                             zh[:, sl])
                    if t + 1 < et:
                        hT_new = st.tile([128, H], BF16, tag="hT")
                        for half in range(2):
                            p = ptr.tile([128, 256], BF16, tag="tr")
                            for c in range(2):
                                cc = 2 * half + c
                                nc.tensor.transpose(
                                    p[:, 128 * c:128 * (c + 1)],
                                    h_new[:, 128 * cc:128 * (cc + 1)], idb[:])
                            nc.vector.tensor_copy(
                                hT_new[:, 256 * half:256 * (half + 1)], p[:])
                        hT = hT_new
                    h_b = h_new

                nc.sync.dma_start(cc_in[:], h_b[:])

            # ================= RESHARD =================
            nc.gpsimd.collective_compute(
                "AllToAll", OP.bypass,
                replica_groups=[list(range(NCORE))],
                ins=[cc_in[:]], outs=[cc_out[:]])

            with tc.tile_pool(name="pgt", bufs=2, space="PSUM") as pgt, \
                 tc.tile_pool(name="pm1", bufs=1, space="PSUM") as pm1, \
                 tc.tile_pool(name="pm2", bufs=1, space="PSUM") as pm2, \
                 tc.tile_pool(name="ptr2", bufs=2, space="PSUM") as pt2, \
                 tc.tile_pool(name="pdo", bufs=1, space="PSUM") as pdo:

                # ---------- gather + h_x / h_y ----------
                pxa = ch.tile([BD, H], BF16, tag="pxa")
                pxb = ch.tile([BD, H], BF16, tag="pxb")
                pya = ch.tile([BD, H], BF16, tag="pya")
                pyb = ch.tile([BD, H], BF16, tag="pyb")
                nc.sync.dma_start(pxa[0:16, :], cc_out[0][:])
                nc.sync.dma_start(pxa[16:32, :], cc_out[1][:])
                nc.sync.dma_start(pxb[0:16, :], cc_out[2][:])
                nc.sync.dma_start(pxb[16:32, :], cc_out[3][:])
                nc.sync.dma_start(pya[0:16, :], cc_out[4][:])
                nc.sync.dma_start(pya[16:32, :], cc_out[5][:])
                nc.sync.dma_start(pyb[0:16, :], cc_out[6][:])
                nc.sync.dma_start(pyb[16:32, :], cc_out[7][:])
                hx = pe.tile([BD, H], BF16, tag="hx")
                hy = pe.tile([BD, H], BF16, tag="hy")
                nc.vector.tensor_add(hx[:], pxa[:], pxb[:])
                nc.vector.tensor_add(hy[:], pya[:], pyb[:])

                def trsp32(src_bf, tag):
                    """src [32, 512] (partition base 0) -> [128, 128] bf16,
                    h-dim chunk c -> cols 32c:32c+32."""
                    p = pt2.tile([128, 128], BF16, tag="trp")
                    for c in range(4):
                        nc.tensor.transpose(
                            p[:, 32 * c:32 * (c + 1)],
                            src_bf[0:32, 128 * c:128 * (c + 1)],
                            idb[0:32, 0:32])
                    o = pe.tile([128, 128], BF16, tag=tag)
                    nc.scalar.copy(o[:], p[:])
                    return o

                hxT = trsp32(hx, "hxT")
                hyT = trsp32(hy, "hyT")

                # ---------- m1 = relu(W1x hx + W1y hy + b1), blocked ------
                m1 = pm1.tile([128, 256], F32, tag="m1")
                for j in range(4):
                    for k in range(4):
                        nc.tensor.matmul(
                            m1[32 * j:32 * (j + 1), :], hxT[:, 32 * k:32 * (k + 1)],
                            em1x[k][:, 256 * j:256 * (j + 1)],
                            start=(k == 0), stop=False,
                            tile_position=(0, 32 * j))
                    for k in range(4):
                        nc.tensor.matmul(
                            m1[32 * j:32 * (j + 1), :], hyT[:, 32 * k:32 * (k + 1)],
                            em1y[k][:, 256 * j:256 * (j + 1)],
                            start=False, stop=False,
                            tile_position=(0, 32 * j))
                    nc.tensor.matmul(
                        m1[32 * j:32 * (j + 1), :], ones_b[0:1, 0:BD],
                        em1b[:, 256 * j:256 * (j + 1)], start=False, stop=True,
                        tile_position=(0, 32 * j))
                hm1 = ch.tile([128, 256], BF16, tag="hm1")
                nc.scalar.activation(hm1[:], m1[:], AF.Relu)

                def tr_m1(src_bf, tag):
                    """blocked [ (j,b32), 256 ] -> [128, 32*8] chunks of the
                    1024-dim; chunk k at dst col 32*perm[k]. Two full-tile
                    half transposes: left half -> even chunks, right -> odd."""
                    p = pt2.tile([128, 256], BF16, tag="trp")
                    nc.tensor.transpose(p[:, 0:128], src_bf[:, 0:128], idb[:])
                    nc.tensor.transpose(p[:, 128:256], src_bf[:, 128:256],
                                        idb[:])
                    perm = [0, 4, 1, 5, 2, 6, 3, 7]
                    o = pe.tile([128, 256], BF16, tag=tag)
                    nc.scalar.copy(o[:], p[:])
                    return o, perm

                hm1T, p1 = tr_m1(hm1, "hm1T")

                # ---------- m2 = relu(W2 m1 + b2), blocked ----------------
                m2 = pm2.tile([128, 128], F32, tag="m2")
                for j in range(4):
                    for k in range(8):
                        nc.tensor.matmul(
                            m2[32 * j:32 * (j + 1), :],
                            hm1T[:, 32 * p1[k]:32 * (p1[k] + 1)],
                            em2[k][:, 128 * j:128 * (j + 1)],
                            start=(k == 0), stop=False,
                            tile_position=(0, 32 * j))
                    nc.tensor.matmul(
                        m2[32 * j:32 * (j + 1), :], ones_b[0:1, 0:BD],
                        em2b[:, 128 * j:128 * (j + 1)], start=False, stop=True,
                        tile_position=(0, 32 * j))
                hm2 = ch.tile([128, 128], BF16, tag="hm2")
                nc.scalar.activation(hm2[:], m2[:], AF.Relu)

                def tr_sq(src_bf, tag):
                    """blocked [(j,b32), 128] -> [128, 128]; chunk j -> col 32j.
                    One full-tile transpose: out[:, 32j+b] = src[32j+b, :]."""
                    p = pt2.tile([128, 128], BF16, tag="trp")
                    nc.tensor.transpose(p[:], src_bf[:], idb[:])
                    o = pe.tile([128, 128], BF16, tag=tag)
                    nc.scalar.copy(o[:], p[:])
                    return o

                hm2T = tr_sq(hm2, "hm2T")

                # ---------- z = eo m2 + b, blocked ------------------------
                zp = pm2.tile([128, 128], F32, tag="m2")
                for j in range(4):
                    for k in range(4):
                        nc.tensor.matmul(
                            zp[32 * j:32 * (j + 1), :],
                            hm2T[:, 32 * k:32 * (k + 1)],
                            eo[k][:, 128 * j:128 * (j + 1)],
                            start=(k == 0), stop=False,
                            tile_position=(0, 32 * j))
                    nc.tensor.matmul(
                        zp[32 * j:32 * (j + 1), :], ones_b[0:1, 0:BD],
                        eob[:, 128 * j:128 * (j + 1)], start=False, stop=True,
                        tile_position=(0, 32 * j))
                z_b = ch.tile([128, 128], BF16, tag="z_b")
                nc.scalar.copy(z_b[:], zp[:])
                zT = tr_sq(z_b, "zT")

                # ---------- cst = dcw [hx; z] + dcb, blocked --------------
                # cst stays in SBUF: per-row constant for every decode step's
                # gates, applied via id32-stationary matmuls.
                cstp = pgt.tile([128, 512], F32, tag="g")
                for j in range(4):
                    for k in range(4):
                        nc.tensor.matmul(
                            cstp[32 * j:32 * (j + 1), 0:384],
                            hxT[:, 32 * k:32 * (k + 1)],
                            dcw[k][:, 384 * j:384 * (j + 1)],
                            start=(k == 0), stop=False,
                            tile_position=(0, 32 * j))
                    for k in range(4):
                        nc.tensor.matmul(
                            cstp[32 * j:32 * (j + 1), 0:384],
                            zT[:, 32 * k:32 * (k + 1)],
                            dcw[4 + k][:, 384 * j:384 * (j + 1)],
                            start=False, stop=False,
                            tile_position=(0, 32 * j))
                    nc.tensor.matmul(
                        cstp[32 * j:32 * (j + 1), 0:384], ones_b[0:1, 0:BD],
                        dcb[:, 384 * j:384 * (j + 1)], start=False, stop=True,
                        tile_position=(0, 32 * j))
                # cst stored [32, 4*384]: block j at cols 384j (partition base
                # 0, so it can be a matmul stationary alongside id32).
                cst_b = pe.tile([BD, G], BF16, tag="cst")
                for j in range(4):
                    nc.scalar.copy(cst_b[0:32, 384 * j:384 * (j + 1)],
                                   cstp[32 * j:32 * (j + 1), 0:384])

                # ---------- decoder init ----------
                hd_b = st.tile([128, 128], BF16, tag="hd")
                nc.gpsimd.memset(hd_b[:], 0.0)

                id32 = idb[0:32, 0:32]

                def emit_const(g):
                    """cst contribution: rz part -> cols 0:256 (accumulate),
                    bih_n part -> cols 384:512 (starts that region)."""
                    for j in range(4):
                        nc.tensor.matmul(
                            g[32 * j:32 * (j + 1), 0:256], id32,
                            cst_b[0:32, 384 * j:384 * j + 256],
                            start=False, stop=False,
                            tile_position=(0, 32 * j))
                    for j in range(4):
                        nc.tensor.matmul(
                            g[32 * j:32 * (j + 1), 384:512], id32,
                            cst_b[0:32, 384 * j + 256:384 * (j + 1)],
                            start=True, stop=False,
                            tile_position=(0, 32 * j))

                def emit_hside(g, hdT_src):
                    """Whh (cols 0:384, starts 0:384) + bhh_n bias (stops
                    256:384)."""
                    for k in range(4):
                        for j in range(4):
                            nc.tensor.matmul(
                                g[32 * j:32 * (j + 1), 0:384],
                                hdT_src[:, 32 * k:32 * (k + 1)],
                                dwhh[k][:, 384 * j:384 * (j + 1)],
                                start=(k == 0), stop=False,
                                tile_position=(0, 32 * j))
                    for j in range(4):
                        nc.tensor.matmul(
                            g[32 * j:32 * (j + 1), 256:384], ones_b[0:1, 0:BD],
                            dbhhn[:, 128 * j:128 * (j + 1)],
                            start=False, stop=True,
                            tile_position=(0, 32 * j))

                def emit_fused(g, hm2T_src):
                    """hm2-side (fused do->Wy): rz waves first (close 0:256),
                    then xn waves (close 384:512)."""
                    for k in range(4):
                        for j in range(4):
                            nc.tensor.matmul(
                                g[32 * j:32 * (j + 1), 0:256],
                                hm2T_src[:, 32 * k:32 * (k + 1)],
                                wfz[k][:, 256 * j:256 * (j + 1)],
                                start=False, stop=(k == 3),
                                tile_position=(0, 32 * j))
                    for k in range(4):
                        for j in range(4):
                            nc.tensor.matmul(
                                g[32 * j:32 * (j + 1), 384:512],
                                hm2T_src[:, 32 * k:32 * (k + 1)],
                                wfn[k][:, 128 * j:128 * (j + 1)],
                                start=False, stop=(k == 3),
                                tile_position=(0, 32 * j))

                P1 = [0, 4, 1, 5, 2, 6, 3, 7]

                # ---- step-0 gates: cst + x_last-side (host-precomputed) ----
                g_cur = pgt.tile([128, 512], F32, tag="g")
                for j in range(4):
                    nc.tensor.matmul(
                        g_cur[32 * j:32 * (j + 1), 0:256], id32,
                        cst_b[0:32, 384 * j:384 * j + 256],
                        start=True, stop=False, tile_position=(0, 32 * j))
                for j in range(4):
                    nc.tensor.matmul(
                        g_cur[32 * j:32 * (j + 1), 0:256], id32,
                        xg0[0:32, 384 * j:384 * j + 256],
                        start=False, stop=True, tile_position=(0, 32 * j))
                for j in range(4):
                    nc.tensor.matmul(
                        g_cur[32 * j:32 * (j + 1), 384:512], id32,
                        cst_b[0:32, 384 * j + 256:384 * (j + 1)],
                        start=True, stop=False, tile_position=(0, 32 * j))
                for j in range(4):
                    nc.tensor.matmul(
                        g_cur[32 * j:32 * (j + 1), 384:512], id32,
                        xg0[0:32, 384 * j + 256:384 * (j + 1)],
                        start=False, stop=True, tile_position=(0, 32 * j))
                for j in range(4):
                    nc.tensor.matmul(
                        g_cur[32 * j:32 * (j + 1), 256:384], ones_b[0:1, 0:BD],
                        dbhhn[:, 128 * j:128 * (j + 1)],
                        start=True, stop=True, tile_position=(0, 32 * j))

                # ================= DECODE =================
                for t in range(hor):
                    g = g_cur
                    # ---- GRU cell elementwise ----
                    rz = ch.tile([128, 256], BF16, tag="rz")
                    nc.scalar.activation(rz[:], g[:, 0:256], AF.Sigmoid)
                    # off-path (GpSimd): omz = 1-z = sigmoid(-g_z); zh = z*h
                    omz = ch.tile([128, 128], BF16, tag="omz")
                    nc.vector.tensor_scalar(omz[:], rz[:, 128:256],
                                            -1.0, 1.0, OP.mult, OP.add)
                    zh = ch.tile([128, 128], BF16, tag="zh")
                    nc.vector.tensor_mul(zh[:], rz[:, 128:256], hd_b[:])
                    # n-path
                    rhn = ch.tile([128, 128], BF16, tag="drhn")
                    nc.vector.tensor_mul(rhn[:], rz[:, 0:128], g[:, 256:384])
                    npre = ch.tile([128, 128], BF16, tag="dnpre")
                    nc.vector.tensor_add(npre[:], rhn[:], g[:, 384:512])
                    n_b = ch.tile([128, 128], BF16, tag="dn")
                    nc.scalar.activation(n_b[:], npre[:], AF.Tanh)
                    # h' = omz*n + zh
                    t1 = ch.tile([128, 128], BF16, tag="dt1")
                    nc.vector.tensor_mul(t1[:], omz[:], n_b[:])
                    hd_new = st.tile([128, 128], BF16, tag="hd")
                    nc.vector.tensor_add(hd_new[:], t1[:], zh[:])

                    ptd = pt2.tile([128, 128], BF16, tag="trp")
                    nc.tensor.transpose(ptd[:], hd_new[:], idb[:])
                    hdT = st.tile([128, 128], BF16, tag="hdT")
                    nc.scalar.copy(hdT[:], ptd[:])
                    hd_b = hd_new

                    # ---- m1 ----
                    # bias wave opens the bank (weakest deps, hoisted off
                    # the critical path); the k3 data wave closes it
                    m1 = pm1.tile([128, 256], F32, tag="m1")
                    for j in range(4):
                        nc.tensor.matmul(
                            m1[32 * j:32 * (j + 1), :], ones_b[0:1, 0:BD],
                            dm1b[:, 256 * j:256 * (j + 1)],
                            start=True, stop=False,
                            tile_position=(0, 32 * j))
                    for k in range(4):
                        for j in range(4):
                            nc.tensor.matmul(
                                m1[32 * j:32 * (j + 1), :],
                                hdT[:, 32 * k:32 * (k + 1)],
                                dm1[k][:, 256 * j:256 * (j + 1)],
                                start=False, stop=(k == 3),
                                tile_position=(0, 32 * j))
                    # pre-emit next step's h-side gates: they stream on PE
                    # while this step's m1 relu/transpose waits on Scalar.
                    if t + 1 < hor:
                        g_cur = pgt.tile([128, 512], F32, tag="g")
                        emit_hside(g_cur, hdT)
                        emit_const(g_cur)

                    hm1 = ch.tile([128, 256], BF16, tag="hm1")
                    p1t = pt2.tile([128, 256], BF16, tag="trp")
                    hm1T = ch.tile([128, 256], BF16, tag="hm1T")
                    for hf in range(2):
                        sl = slice(128 * hf, 128 * (hf + 1))
                        nc.scalar.activation(hm1[:, sl], m1[:, sl], AF.Relu)
                        nc.tensor.transpose(p1t[:, sl], hm1[:, sl], idb[:])
                        nc.vector.tensor_copy(hm1T[:, sl], p1t[:, sl])

                    # ---- m2 (even chunks first: left half of hm1T) ----
                    m2 = pm2.tile([128, 128], F32, tag="m2")
                    for j in range(4):
                        nc.tensor.matmul(
                            m2[32 * j:32 * (j + 1), :], ones_b[0:1, 0:BD],
                            dm2b[:, 128 * j:128 * (j + 1)],
                            start=True, stop=False,
                            tile_position=(0, 32 * j))
                    for ki, k in enumerate([0, 2, 4, 6, 1, 3, 5, 7]):
                        for j in range(4):
                            nc.tensor.matmul(
                                m2[32 * j:32 * (j + 1), :],
                                hm1T[:, 32 * P1[k]:32 * (P1[k] + 1)],
                                dm2[k][:, 128 * j:128 * (j + 1)],
                                start=False, stop=(ki == 7),
                                tile_position=(0, 32 * j))
                    hm2 = ch.tile([128, 128], BF16, tag="hm2")
                    nc.scalar.activation(hm2[:], m2[:], AF.Relu)
                    p2t = pt2.tile([128, 128], BF16, tag="trp")
                    nc.tensor.transpose(p2t[:], hm2[:], idb[:])
                    hm2T = ch.tile([128, 128], BF16, tag="hm2T")
                    nc.vector.tensor_copy(hm2T[:], p2t[:])

                    # ---- fused y-side of next step's gates (critical path) --
                    if t + 1 < hor:
                        emit_fused(g_cur, hm2T)

                    # ---- output head (off critical path) ----
                    ytp = pdo.tile([NY, BD], F32, tag="do")
                    for k in range(4):
                        nc.tensor.matmul(ytp[:], dow[k][:, 0:NY],
                                         hm2T[:, 32 * k:32 * (k + 1)],
                                         start=(k == 0), stop=False)
                    nc.tensor.matmul(ytp[:], dob[:], ones_b[0:1, 0:BD],
                                     start=False, stop=True)
                    y_sb = ch.tile([NY, BD], F32, tag="y_sb")
                    nc.vector.tensor_copy(y_sb[:], ytp[:])
                    nc.sync.dma_start(d_out[:, BD * t:BD * (t + 1)], y_sb[:])

    nc.compile()
    return nc


# ---------------------------------------------------------------------------
# Host-side sharding
# ---------------------------------------------------------------------------

# gate-block permutation: blocked col j*384 + s*128 + c  <- gate row 512s+128j+c
_IDXG = np.array([512 * s + 128 * j + c
                  for j in range(4) for s in range(3) for c in range(128)])
_IDXRZ = np.array([512 * s + 128 * j + c
                   for j in range(4) for s in range(2) for c in range(128)])
_IDXN = np.array([1024 + 128 * j + c for j in range(4) for c in range(128)])
# step-0 gate blocked layout: per block j, [rz(256) | xn(128)]
_IDXG0 = np.array([(512 * (u // 128) + 128 * j + (u % 128)) if u < 256
                   else (1024 + 128 * j + (u - 256))
                   for j in range(4) for u in range(384)])


def shard_inputs(inp, et=ET, hor=HOR):
    f32 = np.float32
    x, y = np.asarray(inp["x"], f32), np.asarray(inp["y"], f32)
    chains = [("xf", False, x), ("xb", True, x),
              ("ef", False, y), ("eb", True, y)]
    in_maps = []
    shared = {}

    def bf(a):
        return np.ascontiguousarray(np.asarray(a, f32).astype(BF))

    def wih_aug(pre):
        wih = np.asarray(inp[pre + "_Wih"], f32)
        bih = np.asarray(inp[pre + "_bih"], f32)
        bhh = np.asarray(inp[pre + "_bhh"], f32)
        aug = np.zeros((66, G), f32)
        aug[0:64, :] = wih.T
        bias = bih.copy()
        bias[0:2 * H] += bhh[0:2 * H]
        aug[64, :] = bias
        aug[65, H:2 * H] = BIG
        return bf(aug)

    d_Wih = np.asarray(inp["d_Wih"], f32)
    d_bih = np.asarray(inp["d_bih"], f32)
    d_bhh = np.asarray(inp["d_bhh"], f32)
    do_W = np.asarray(inp["do_W"], f32)
    do_b = np.asarray(inp["do_b"], f32)

    # decoder GRU weights, gate-blocked
    shared["dwhh_blk"] = bf(np.asarray(inp["d_Whh"], f32).T[:, _IDXG])
    WyT = d_Wih[:, 2 * H:].T                      # [64, 1536]
    # fused do->Wy: y_p-side of gates = hm2 @ Wfuse + bfuse
    Wfuse = do_W.T @ WyT                           # [512, 1536]
    bfuse = do_b @ WyT                             # [1536]
    shared["wfz_blk"] = bf(Wfuse[:, _IDXRZ])
    shared["wfn_blk"] = bf(Wfuse[:, _IDXN])
    shared["dbhhn_blk"] = bf(d_bhh[None, 2 * H:])
    dcb = d_bih + bfuse
    dcb[0:2 * H] += d_bhh[0:2 * H]
    shared["dcw_blk"] = bf(d_Wih[:, 0:2 * H].T[:, _IDXG])
    shared["dcb_blk"] = bf(dcb[None, _IDXG])

    shared["dm1_t"] = bf(np.asarray(inp["dm_W1"], f32).T)
    shared["dm1b_row"] = bf(np.asarray(inp["dm_b1"], f32)[None, :])
    shared["dm2_t"] = bf(np.asarray(inp["dm_W2"], f32).T)
    shared["dm2b_row"] = bf(np.asarray(inp["dm_b2"], f32)[None, :])
    shared["dow_t"] = bf(do_W.T)
    shared["dob_row"] = bf(do_b[None, :])

    em_W1 = np.asarray(inp["em_W1"], f32)
    shared["em1x_t"] = bf(em_W1[:, 0:H].T)
    shared["em1y_t"] = bf(em_W1[:, H:].T)
    shared["em1b_row"] = bf(np.asarray(inp["em_b1"], f32)[None, :])
    shared["em2_t"] = bf(np.asarray(inp["em_W2"], f32).T)
    shared["em2b_row"] = bf(np.asarray(inp["em_b2"], f32)[None, :])
    shared["eo_t"] = bf(np.asarray(inp["eo_W"], f32).T)
    shared["eob_row"] = bf(np.asarray(inp["eo_b"], f32)[None, :])

    for j in range(NCORE):
        chain, half = j // 2, j % 2
        pre, rev, seq = chains[chain]
        T = seq.shape[1]
        s = seq[128 * half:128 * (half + 1)]          # [128, T, 64]
        xin = np.zeros((66, et, BE), f32)
        xin[64, :, :] = 1.0
        pad = et - T
        if pad:
            xin[65, 0:pad, :] = 1.0
        order = np.arange(T)[::-1] if rev else np.arange(T)
        xin[0:64, pad:, :] = s[:, order, :].transpose(2, 1, 0)
        m = dict(shared)
        m["xin"] = bf(xin.reshape(66, et * BE))
        m["wih_aug"] = wih_aug(pre)
        m["whh_t"] = bf(np.asarray(inp[pre + "_Whh"], f32).T)
        m["bhhn_row"] = bf(np.asarray(inp[pre + "_bhh"], f32)[None, 2 * H:])
        xl = np.concatenate([x[16 * j:16 * j + 16, -1, :],
                             x[128 + 16 * j:128 + 16 * j + 16, -1, :]])
        # bfuse is folded into cst (applied every step), but step 0's y-side
        # is x_last directly (no do_b) -> pre-subtract it here.
        xg0 = xl @ WyT - bfuse                        # [32, 1536]
        m["xg0_blk"] = bf(xg0[:, _IDXG0])
        in_maps.append(m)
    return in_maps


def unshard(results, hor=HOR):
    out = np.zeros((B, hor, NY), np.float32)
    for j in range(NCORE):
        o = results[j]["out"].reshape(NY, hor, BD).transpose(2, 1, 0)
        out[16 * j:16 * j + 16] = o[0:16]
        out[128 + 16 * j:128 + 16 * j + 16] = o[16:32]
    return out


_NC = None


def kernel(**inputs):
    global _NC
    from concourse.bass_utils import run_bass_kernel_spmd
    if _NC is None:
        _NC = build_nc()
    in_maps = shard_inputs(inputs)
    res = run_bass_kernel_spmd(_NC, in_maps, core_ids=list(range(NCORE)))
    return unshard(res.results)
